# revision 61
# baseline (speedup 1.0000x reference)
"""Multi-head attention (b=2, sq=skv=2048, dim=1024, 16 heads x 64) on 8 TRN2
NeuronCores.

Sharding: 2 heads per core (head-parallel), with the matching tensor-parallel
column slice of W_qkv and row slice of W_out.  Each core computes a partial
output projection over its 128 head-dims; the all-reduce of the 8 partials
(+ bias) happens on the host during unshard.

Per-core kernel (bf16 compute, fp32 PSUM accumulation):
  phase 1: qT/kT/vT = W.T @ x.T ([128 = 2 heads x 64 dims, tokens]); v is
           additionally PE-transposed to natural [token, dim] layout with a
           ones column appended (denominator trick).
  phase 2: per (batch, q-tile, k-tile): scoresT for both heads ([k-tokens, q])
           in one 2-bank PSUM group; one exp ACTIVATE over the group (scale
           1/8 fused, no max subtraction -- scores range +-10); PV matmuls
           accumulate [v | 1].T @ expT over the 16 k-tiles giving unnormalized
           outT plus the softmax denominator in row 64.
  flush:   per (batch, q-tile): DVE reciprocal reads the two denominator rows
           straight out of the PSUM accumulators; a single f32r outer-product
           matmul (selector [2,128] x recips [2,512]) broadcasts both heads'
           reciprocals across the 128 partitions; two DVE multiplies read the
           accumulators directly from PSUM and write normalized bf16 outT.
  phase 3: partial out = outT.T @ W_out_rows -> bf16 [tokens, 1024].

DMA: weights (contiguous per-partition layout) + q chunks ride the scalar
HWDGE queue, kv chunks + outputs the sync HWDGE queue, so the startup
critical path (wk -> kv0 -> kproj0 -> qproj0 -> first scores) is ~7us
shorter than a single-queue layout.  Emission is a hand-tuned interleave:
the dependency-driven Tile scheduler always has dep-free PE work (projection
chunks, out-projection quarters) inside the ACT(exp)-bound attention stream,
with out-projection pieces reserved as PE filler for the tail flushes.
"""

import os
import sys

for _p in ("/opt/trn_rl_repo", "/root/.axon_site/_ro/trn_rl_repo"):
    if os.path.isdir(_p) and _p not in sys.path:
        sys.path.append(_p)

import ml_dtypes
import numpy as np

import concourse.bass as bass  # noqa: F401
import concourse.tile as tile
from concourse import bacc, mybir
from concourse.bass_utils import run_bass_kernel_spmd
from concourse.masks import make_identity

B, SQ, SKV, DIM = 2, 2048, 2048, 1024
HEADS, DH = 16, 64
N_CORES = 8
HPC = HEADS // N_CORES  # heads per core = 2
HD = HPC * DH  # 128 head-dim rows per core
TOK = B * SQ  # 4096
KO = DIM // 128  # 8 contraction chunks of 128
SCALE = DH**-0.5

BF16 = mybir.dt.bfloat16
F32 = mybir.dt.float32

PCHUNK = 512  # token chunk in projections (contiguous per-chunk dram layout)
QTILE = 512  # q tile in attention
KTILE = 128  # k tile (scores psum partition dim)
NKT = SKV // KTILE  # 16
NQT = SQ // QTILE  # 4
NCH = SQ // PCHUNK  # 4 chunks per batch

BF = ml_dtypes.bfloat16
Exp = mybir.ActivationFunctionType.Exp

LOOKAHEAD = 6


def build():
    nc = bacc.Bacc(
        "TRN2", target_bir_lowering=False, debug=False, num_devices=N_CORES
    )

    xqt_d = nc.dram_tensor(
        "xqt", [B * NCH, 128, KO, PCHUNK], BF16, kind="ExternalInput"
    )
    xkvt_d = nc.dram_tensor(
        "xkvt", [B * NCH, 128, KO, PCHUNK], BF16, kind="ExternalInput"
    )
    # weights pre-rearranged on host to partition-major contiguous layout,
    # packed into two bundles so each is one large efficient DMA:
    # wkq = [wk | wq], wvo = [wv | wout]
    wkq_d = nc.dram_tensor("wkq", [128, 2, KO, HD], BF16, kind="ExternalInput")
    wvo_d = nc.dram_tensor("wvo", [128, 2, KO, HD], BF16, kind="ExternalInput")
    out_d = nc.dram_tensor("out", [TOK, DIM], BF16, kind="ExternalOutput")

    xqt = xqt_d.ap()
    xkvt = xkvt_d.ap()
    F32R = mybir.dt.float32r

    with tile.TileContext(nc) as tc:
        with (
            tc.tile_pool(name="persist", bufs=1) as persist,
            tc.tile_pool(name="xin", bufs=8) as xin,
            tc.tile_pool(name="exps", bufs=6) as exps,
            tc.tile_pool(name="ost", bufs=3) as ost,
            tc.tile_pool(name="nrm", bufs=2) as nrm,
            tc.tile_pool(name="pkp", bufs=2) as pkp,
            tc.tile_pool(name="spsum", bufs=2, space="PSUM") as spsum,
            tc.tile_pool(name="accp", bufs=2, space="PSUM") as accp,
            tc.tile_pool(name="miscp", bufs=2, space="PSUM") as miscp,
            tc.tile_pool(name="drp", bufs=2, space="DRAM") as drp,
        ):
            # --- startup loads spread across the three DMA queues so the
            # first scores land ~8us earlier than a serial layout ---
            xts = {}  # (b, c) -> loaded x chunk tile (or list of sub-tiles)

            # Startup loads in consumption-priority order.  The sync HWDGE
            # queue is the fast one (~260-320 GB/s), the gpsimd SWDGE queue
            # is mid (~190), and the scalar HWDGE queue crawls (~50-90, the
            # low-priority weights queue) so it only carries loose-deadline
            # mid-kernel chunks.
            wkq_sb = persist.tile([128, 2, KO, HD], BF16, tag="wkq")
            nc.sync.dma_start(wkq_sb[:], wkq_d.ap())
            wk_sb = wkq_sb[:, 0]
            wq_sb = wkq_sb[:, 1]

            def kv_load(b, c, eng=None):
                t = xin.tile([128, KO, PCHUNK], BF16, tag="x")
                (eng or nc.sync).dma_start(t[:], xkvt[b * NCH + c])
                xts[(b, c)] = t

            def q_load(b, c, eng=None):
                t = xin.tile([128, KO, PCHUNK], BF16, tag="x")
                (eng or nc.gpsimd).dma_start(t[:], xqt[b * NCH + c])
                xts[("q", b, c)] = t

            kv_load(0, 0, nc.sync)
            kv_load(0, 1, nc.gpsimd)
            q_load(0, 0, nc.sync)
            wvo_sb = persist.tile([128, 2, KO, HD], BF16, tag="wvo")
            nc.gpsimd.dma_start(wvo_sb[:], wvo_d.ap())
            wv_sb = wvo_sb[:, 0]
            wout_sb = wvo_sb[:, 1].rearrange("p a b -> p (a b)")
            kv_load(0, 2, nc.sync)
            kv_load(0, 3, nc.gpsimd)
            q_load(0, 1, nc.sync)

            # --- constants ---
            ident = persist.tile([128, DH], BF16, tag="ident")
            make_identity(nc, ident[0:DH, :])
            make_identity(nc, ident[DH : 2 * DH, :])
            # head-selector for the reciprocal broadcast: partition h*64
            # covers output partitions [h*64, (h+1)*64); rows 1-63 stay zero
            st2 = persist.tile([DH + 1, 128], F32, tag="st2")
            nc.vector.memset(st2[:], 0.0)
            nc.vector.memset(st2[0:1, 0:DH], 1.0)
            nc.vector.memset(st2[DH : DH + 1, DH : 2 * DH], 1.0)
            # reciprocal staging at partitions {0, 64}; rows 1-63 are zeroed
            # once and never written again, keeping the f32r contraction exact
            rcps = []
            for i in range(2):
                r = persist.tile([DH + 1, QTILE], F32, tag=f"rcp{i}")
                nc.vector.memset(r[:], 0.0)
                rcps.append(r)
            # prefetch the exp table set during the initial DMAs
            dummy = persist.tile([1, 8], F32, tag="dummy")
            nc.vector.memset(dummy[:], 0.0)
            nc.scalar.activation(dummy[:], dummy[:], Exp)

            qt_sb, kt_sb, vt_sb, vnat, outT = {}, {}, {}, {}, {}
            for b in range(B):
                qt_sb[b] = persist.tile([HD, SQ], BF16, tag=f"qt{b}", name=f"qt{b}")
                kt_sb[b] = persist.tile([HD, SKV], BF16, tag=f"kt{b}", name=f"kt{b}")
                vt_sb[b] = persist.tile([HD, SKV], BF16, tag=f"vt{b}", name=f"vt{b}")
                vnat[b] = persist.tile(
                    [128, HPC, NKT, DH + 1], BF16, tag=f"vn{b}", name=f"vn{b}"
                )
                outT[b] = persist.tile([HD, SQ], BF16, tag=f"ot{b}", name=f"ot{b}")
                nc.vector.memset(vnat[b][:, :, :, DH], 1.0)

            def _proj(dst, w_sb, xt, t0, w=PCHUNK):
                ps = miscp.tile([128, w], F32, tag="m", name="projp")
                for ko in range(KO):
                    nc.tensor.matmul(
                        ps[:],
                        w_sb[:, ko, :],
                        xt[:, ko, :],
                        start=(ko == 0),
                        stop=(ko == KO - 1),
                    )
                nc.vector.tensor_copy(dst[:, t0 : t0 + w], ps[:])

            def kproj(b, c):
                _proj(kt_sb[b], wk_sb, xts[(b, c)], c * PCHUNK)

            def qproj(b, c):
                _proj(qt_sb[b], wq_sb, xts.pop(("q", b, c)), c * PCHUNK)

            def vproj(b, c):
                """V projection for chunk c + PE-transpose into natural
                layout (k-tiles 4c..4c+3); frees the x chunk tile."""
                _proj(vt_sb[b], wv_sb, xts.pop((b, c)), c * PCHUNK)
                for h in range(HPC):
                    tp = miscp.tile([128, 4, DH], BF16, tag="m", name="vtp")
                    for i in range(4):
                        j = c * 4 + i
                        nc.tensor.transpose(
                            tp[:, i, :],
                            vt_sb[b][
                                h * DH : (h + 1) * DH,
                                j * KTILE : (j + 1) * KTILE,
                            ],
                            ident[h * DH : (h + 1) * DH, :],
                        )
                    nc.vector.tensor_copy(
                        vnat[b][:, h, c * 4 : (c + 1) * 4, 0:DH], tp[:]
                    )

            # --- attention ---
            acc_store = {0: {}, 1: {}}
            ucps = {}

            def attention(b, pre, post):
                """Flat software-pipelined attention over all (qt, j) steps.

                Scores for step t+LOOKAHEAD are emitted before PV of step t,
                so the PE always has score matmuls queued ahead of the
                exp/PV chain.  pre[t] hooks fire before scores(t); post[s]
                hooks fire right after step s's PV matmuls.
                """
                NT = NQT * NKT
                sps = {}
                accs = acc_store[b]

                def emit_scores(t):
                    qt, j = divmod(t, NKT)
                    q_sl = slice(qt * QTILE, (qt + 1) * QTILE)
                    k_sl = slice(j * KTILE, (j + 1) * KTILE)
                    sp = spsum.tile([128, HPC, QTILE], F32, tag="s", name="sp")
                    sps[t] = sp
                    for h in range(HPC):
                        h_sl = slice(h * DH, (h + 1) * DH)
                        nc.tensor.matmul(
                            sp[:, h, :],
                            kt_sb[b][h_sl, k_sl],
                            qt_sb[b][h_sl, q_sl],
                            start=True,
                            stop=True,
                        )

                def emit_tail(t):
                    qt, j = divmod(t, NKT)
                    sp = sps.pop(t)
                    ex = exps.tile([128, HPC, QTILE], BF16, tag="e", name="ex")
                    nc.scalar.activation(ex[:], sp[:], Exp, scale=SCALE)
                    if j == 0:
                        accs[qt] = [
                            accp.tile([128, QTILE], F32, tag="acc", name="acc")
                            for _ in range(HPC)
                        ]
                    for h in range(HPC):
                        nc.tensor.matmul(
                            accs[qt][h][0 : DH + 1, :],
                            vnat[b][:, h, j, :],
                            ex[:, h, :],
                            start=(j == 0),
                            stop=(j == NKT - 1),
                        )
                    if j == NKT - 1:
                        # drain the PSUM accumulators (incl. denominator row)
                        # to SBUF immediately; normalization is deferred
                        ucp = nrm.tile(
                            [DH + 1, HPC, QTILE], F32, tag="u", name="ucp"
                        )
                        acc2 = accs.pop(qt)
                        for h in range(HPC):
                            nc.vector.tensor_copy(
                                ucp[:, h, :], acc2[h][0 : DH + 1, :]
                            )
                        ucps[(b, qt)] = ucp

                for t in range(NT + LOOKAHEAD):
                    for fn in pre.get(t, ()):
                        fn()
                    if t < NT:
                        emit_scores(t)
                    if t >= LOOKAHEAD:
                        emit_tail(t - LOOKAHEAD)
                        for fn in post.get(t - LOOKAHEAD, ()):
                            fn()

            _flno = [0]
            _flst = {}

            def fstart(b, qt, eng=None):
                """Deferred normalization, part 1: repack the two denominator
                rows [1, 2, 512] -> [128, 8] with an SBUF->SBUF DMA so the
                DVE reciprocal runs wide (a [1, 512] reciprocal costs ~3.4us
                on the DVE; [128, 8] costs ~0.15us), then DMA back to row
                form at partitions {0, 64}.  fapply() runs ~4 steps later so
                the repack latency never stalls the PE.
                """
                ucp = ucps.pop((b, qt))
                rcp = rcps[_flno[0] % 2]
                _flno[0] += 1
                eng = eng or nc.gpsimd
                dpk = pkp.tile([128, HPC * QTILE // 128], F32, tag="dp")
                eng.dma_start(dpk[:], ucp[DH : DH + 1, :, :])
                rpk = pkp.tile([128, HPC * QTILE // 128], F32, tag="rp")
                nc.vector.reciprocal(rpk[:], dpk[:])
                for h in range(HPC):
                    eng.dma_start(
                        rcp[h * DH : h * DH + 1, :],
                        rpk[h * DH : (h + 1) * DH, :],
                    )
                _flst[(b, qt)] = (rcp, ucp)

            def fapply(b, qt):
                """Deferred normalization, part 2: broadcast both heads'
                reciprocals across partitions with a single f32r outer
                product against the head-selector, multiply into bf16 outT."""
                rcp, ucp = _flst.pop((b, qt))
                q_sl = slice(qt * QTILE, (qt + 1) * QTILE)
                bcp = miscp.tile([128, QTILE], F32, tag="m", name="bcp")
                nc.tensor.matmul(
                    bcp[:],
                    st2[:].bitcast(F32R),
                    rcp[:].bitcast(F32R),
                    start=True,
                    stop=True,
                )
                for h in range(HPC):
                    h_sl = slice(h * DH, (h + 1) * DH)
                    nc.vector.tensor_mul(
                        outT[b][h_sl, q_sl], ucp[0:DH, h, :], bcp[h_sl, :]
                    )

            def op(b, tt, split_copy=False):
                """Out-projection for one 128-token chunk + output DMA."""
                t_sl = slice(tt * 128, (tt + 1) * 128)
                ob = ost.tile([128, 2, 512], BF16, tag="o")
                for nt in range(DIM // 512):
                    ps = miscp.tile([128, 512], F32, tag="m", name="projo")
                    nc.tensor.matmul(
                        ps[:],
                        outT[b][:, t_sl],
                        wout_sb[:, nt * 512 : (nt + 1) * 512],
                        start=True,
                        stop=True,
                    )
                    if split_copy and nt % 2 == 0:
                        nc.scalar.copy(ob[:, nt, :], ps[:])
                    else:
                        nc.vector.tensor_copy(ob[:, nt, :], ps[:])
                nc.sync.dma_start(
                    out_d.ap()[
                        b * SQ + tt * 128 : b * SQ + (tt + 1) * 128, :
                    ].rearrange("t (n c) -> t n c", n=2),
                    ob[:],
                )

            # --- startup: first projections, then attention begins ---
            kproj(0, 0)
            qproj(0, 0)

            L = lambda fn, *a, **k: (lambda: fn(*a, **k))

            pre0 = {
                4: [L(kproj, 0, 1)],
                6: [L(vproj, 0, 0)],
                8: [L(kproj, 0, 2)],
                12: [L(kproj, 0, 3)],
            }
            post0 = {
                3: [L(vproj, 0, 1)],
                6: [L(q_load, 0, 2)],
                7: [L(vproj, 0, 2)],
                8: [L(kv_load, 1, 0, nc.scalar)],
                9: [L(qproj, 0, 1)],
                11: [L(vproj, 0, 3)],
                14: [L(q_load, 0, 3)],
                15: [L(kproj, 1, 0)],
                17: [L(qproj, 0, 2)],
                19: [L(kv_load, 1, 1, nc.scalar), L(q_load, 1, 0)],
                23: [L(fstart, 0, 0)],
                25: [L(kproj, 1, 1)],
                27: [L(fapply, 0, 0)],
                28: [L(op, 0, 0)],
                29: [L(op, 0, 1), L(qproj, 0, 3)],
                30: [L(op, 0, 2)],
                31: [L(op, 0, 3)],
                33: [L(kv_load, 1, 2), L(q_load, 1, 1)],
                35: [L(vproj, 1, 0)],
                37: [L(qproj, 1, 0)],
                39: [L(fstart, 0, 1)],
                41: [L(kproj, 1, 2)],
                43: [L(fapply, 0, 1), L(kv_load, 1, 3)],
                44: [L(op, 0, 4)],
                45: [L(op, 0, 5), L(vproj, 1, 1)],
                46: [L(op, 0, 6)],
                47: [L(op, 0, 7)],
                49: [L(q_load, 1, 2)],
                51: [L(qproj, 1, 1)],
                53: [L(kproj, 1, 3)],
                55: [L(fstart, 0, 2)],
                57: [L(vproj, 1, 2), L(q_load, 1, 3)],
                58: [L(vproj, 1, 3)],
                59: [L(fapply, 0, 2)],
                60: [L(op, 0, 8)],
                61: [L(op, 0, 9), L(qproj, 1, 2)],
                62: [L(op, 0, 10)],
                63: [L(op, 0, 11), L(qproj, 1, 3)],
            }
            attention(0, pre0, post0)

            post1 = {
                0: [L(fstart, 0, 3)],
                4: [L(fapply, 0, 3)],
                5: [L(op, 0, 12)],
                7: [L(op, 0, 13)],
                9: [L(op, 0, 14)],
                11: [L(op, 0, 15)],
                17: [L(fstart, 1, 0)],
                21: [L(fapply, 1, 0)],
                22: [L(op, 1, 0)],
                24: [L(op, 1, 1)],
                26: [L(op, 1, 2)],
                28: [L(op, 1, 3)],
                33: [L(fstart, 1, 1)],
                37: [L(fapply, 1, 1)],
                38: [L(op, 1, 4)],
                40: [L(op, 1, 5)],
                42: [L(op, 1, 6)],
                44: [L(op, 1, 7)],
                49: [L(fstart, 1, 2)],
                55: [L(fapply, 1, 2)],
                56: [L(op, 1, 8)],
                58: [L(op, 1, 9)],
            }
            attention(1, {}, post1)
            fstart(1, 3, nc.sync)
            op(1, 10)
            op(1, 11)
            fapply(1, 3)
            op(1, 12, split_copy=True)
            op(1, 13, split_copy=True)
            op(1, 14, split_copy=True)
            op(1, 15, split_copy=True)

    nc.compile()
    return nc


def make_in_maps(x_q, x_kv, W_qkv, W_out):
    x_q = np.asarray(x_q, dtype=np.float32)
    x_kv = np.asarray(x_kv, dtype=np.float32)
    W_qkv = np.asarray(W_qkv, dtype=np.float32)
    W_out = np.asarray(W_out, dtype=np.float32)

    def chunk_tile(x):
        # [TOK, DIM] -> [n_chunks, 128, KO, PCHUNK] with D = ko*128 + p
        xt = x.reshape(TOK, DIM).T.reshape(KO, 128, TOK // PCHUNK, PCHUNK)
        return np.ascontiguousarray(xt.transpose(2, 1, 0, 3)).astype(BF)

    def w_tile(w):
        # [DIM, HD] -> [128, KO, HD] partition-major contiguous
        return np.ascontiguousarray(
            w.reshape(KO, 128, HD).transpose(1, 0, 2)
        ).astype(BF)

    xqt = chunk_tile(x_q)
    xkvt = chunk_tile(x_kv)

    in_maps = []
    for c in range(N_CORES):
        cs = slice(c * HD, (c + 1) * HD)
        wk = w_tile(W_qkv[:, 1024:][:, cs])
        wq = w_tile(W_qkv[:, cs])
        wv = w_tile(W_qkv[:, 2048:][:, cs])
        wout = np.ascontiguousarray(W_out[cs, :]).astype(BF)
        in_maps.append(
            {
                "xqt": xqt,
                "xkvt": xkvt,
                "wkq": np.ascontiguousarray(np.stack([wk, wq], axis=1)),
                "wvo": np.ascontiguousarray(
                    np.stack([wv, wout.reshape(128, KO, HD)], axis=1)
                ),
            }
        )
    return in_maps


def combine(partials, b_out):
    """Sum the 8 per-core partial projections and add the bias."""
    acc = np.zeros((TOK, DIM), dtype=np.float32)
    for p in partials:
        acc += np.asarray(p, dtype=np.float32)
    acc += np.asarray(b_out, dtype=np.float32)
    return acc.reshape(B, SQ, DIM)


_STATE = {}


def _get_nc():
    if "nc" not in _STATE:
        _STATE["nc"] = build()
    return _STATE["nc"]


def run(x_q, x_kv, W_qkv, W_out, b_out, trace=False):
    nc = _get_nc()
    in_maps = make_in_maps(x_q, x_kv, W_qkv, W_out)
    res = run_bass_kernel_spmd(nc, in_maps, list(range(N_CORES)), trace=trace)
    out = combine([r["out"] for r in res.results], b_out)
    return out, res


def kernel(x_q, x_kv, W_qkv, W_out, b_out):
    out, _ = run(x_q, x_kv, W_qkv, W_out, b_out, trace=False)
    return out


# revision 62
# speedup vs baseline: 1.0200x; 1.0200x over previous
"""Multi-head attention (b=2, sq=skv=2048, dim=1024, 16 heads x 64) on 8 TRN2
NeuronCores.

Sharding: 2 heads per core (head-parallel), with the matching tensor-parallel
column slice of W_qkv and row slice of W_out.  Each core computes a partial
output projection over its 128 head-dims; the all-reduce of the 8 partials
(+ bias) happens on the host during unshard.

Per-core kernel (bf16 compute, fp32 PSUM accumulation):
  phase 1: qT/kT/vT = W.T @ x.T ([128 = 2 heads x 64 dims, tokens]); v is
           additionally PE-transposed to natural [token, dim] layout with a
           ones column appended (denominator trick).
  phase 2: per (batch, q-tile, k-tile): scoresT for both heads ([k-tokens, q])
           in one 2-bank PSUM group; one exp ACTIVATE over the group (scale
           1/8 fused, no max subtraction -- scores range +-10); PV matmuls
           accumulate [v | 1].T @ expT over the 16 k-tiles giving unnormalized
           outT plus the softmax denominator in row 64.
  flush:   per (batch, q-tile): DVE reciprocal reads the two denominator rows
           straight out of the PSUM accumulators; a single f32r outer-product
           matmul (selector [2,128] x recips [2,512]) broadcasts both heads'
           reciprocals across the 128 partitions; two DVE multiplies read the
           accumulators directly from PSUM and write normalized bf16 outT.
  phase 3: partial out = outT.T @ W_out_rows -> bf16 [tokens, 1024].

DMA: weights (contiguous per-partition layout) + q chunks ride the scalar
HWDGE queue, kv chunks + outputs the sync HWDGE queue, so the startup
critical path (wk -> kv0 -> kproj0 -> qproj0 -> first scores) is ~7us
shorter than a single-queue layout.  Emission is a hand-tuned interleave:
the dependency-driven Tile scheduler always has dep-free PE work (projection
chunks, out-projection quarters) inside the ACT(exp)-bound attention stream,
with out-projection pieces reserved as PE filler for the tail flushes.
"""

import os
import sys

for _p in ("/opt/trn_rl_repo", "/root/.axon_site/_ro/trn_rl_repo"):
    if os.path.isdir(_p) and _p not in sys.path:
        sys.path.append(_p)

import ml_dtypes
import numpy as np

import concourse.bass as bass  # noqa: F401
import concourse.tile as tile
from concourse import bacc, mybir
from concourse.bass_utils import run_bass_kernel_spmd
from concourse.masks import make_identity

B, SQ, SKV, DIM = 2, 2048, 2048, 1024
HEADS, DH = 16, 64
N_CORES = 8
HPC = HEADS // N_CORES  # heads per core = 2
HD = HPC * DH  # 128 head-dim rows per core
TOK = B * SQ  # 4096
KO = DIM // 128  # 8 contraction chunks of 128
SCALE = DH**-0.5

BF16 = mybir.dt.bfloat16
F32 = mybir.dt.float32

PCHUNK = 512  # token chunk in projections (contiguous per-chunk dram layout)
QTILE = 512  # q tile in attention
KTILE = 128  # k tile (scores psum partition dim)
NKT = SKV // KTILE  # 16
NQT = SQ // QTILE  # 4
NCH = SQ // PCHUNK  # 4 chunks per batch

BF = ml_dtypes.bfloat16
Exp = mybir.ActivationFunctionType.Exp

LOOKAHEAD = 6


def build():
    nc = bacc.Bacc(
        "TRN2", target_bir_lowering=False, debug=False, num_devices=N_CORES
    )

    xqt_d = nc.dram_tensor(
        "xqt", [B * NCH, 128, KO, PCHUNK], BF16, kind="ExternalInput"
    )
    xkvt_d = nc.dram_tensor(
        "xkvt", [B * NCH, 128, KO, PCHUNK], BF16, kind="ExternalInput"
    )
    # weights pre-rearranged on host to partition-major contiguous layout,
    # packed into two bundles so each is one large efficient DMA:
    # wkq = [wk | wq], wvo = [wv | wout]
    wkq_d = nc.dram_tensor("wkq", [128, 2, KO, HD], BF16, kind="ExternalInput")
    wvo_d = nc.dram_tensor("wvo", [128, 2, KO, HD], BF16, kind="ExternalInput")
    out_d = nc.dram_tensor("out", [TOK, DIM], BF16, kind="ExternalOutput")

    xqt = xqt_d.ap()
    xkvt = xkvt_d.ap()
    F32R = mybir.dt.float32r

    with tile.TileContext(nc) as tc:
        with (
            tc.tile_pool(name="persist", bufs=1) as persist,
            tc.tile_pool(name="xin", bufs=8) as xin,
            tc.tile_pool(name="exps", bufs=6) as exps,
            tc.tile_pool(name="ost", bufs=3) as ost,
            tc.tile_pool(name="nrm", bufs=2) as nrm,
            tc.tile_pool(name="pkp", bufs=2) as pkp,
            tc.tile_pool(name="spsum", bufs=2, space="PSUM") as spsum,
            tc.tile_pool(name="accp", bufs=2, space="PSUM") as accp,
            tc.tile_pool(name="miscp", bufs=2, space="PSUM") as miscp,
            tc.tile_pool(name="drp", bufs=2, space="DRAM") as drp,
        ):
            # --- startup loads spread across the three DMA queues so the
            # first scores land ~8us earlier than a serial layout ---
            xts = {}  # (b, c) -> loaded x chunk tile (or list of sub-tiles)

            # Startup loads in consumption-priority order.  The sync HWDGE
            # queue is the fast one (~260-320 GB/s), the gpsimd SWDGE queue
            # is mid (~190), and the scalar HWDGE queue crawls (~50-90, the
            # low-priority weights queue) so it only carries loose-deadline
            # mid-kernel chunks.
            wkq_sb = persist.tile([128, 2, KO, HD], BF16, tag="wkq")
            nc.sync.dma_start(wkq_sb[:], wkq_d.ap())
            wk_sb = wkq_sb[:, 0]
            wq_sb = wkq_sb[:, 1]

            def kv_load(b, c, eng=None):
                t = xin.tile([128, KO, PCHUNK], BF16, tag="x")
                (eng or nc.sync).dma_start(t[:], xkvt[b * NCH + c])
                xts[(b, c)] = t

            def q_load(b, c, eng=None):
                t = xin.tile([128, KO, PCHUNK], BF16, tag="x")
                (eng or nc.gpsimd).dma_start(t[:], xqt[b * NCH + c])
                xts[("q", b, c)] = t

            kv_load(0, 0, nc.sync)
            wvo_sb = persist.tile([128, 2, KO, HD], BF16, tag="wvo")
            nc.gpsimd.dma_start(wvo_sb[:], wvo_d.ap())
            wv_sb = wvo_sb[:, 0]
            wout_sb = wvo_sb[:, 1].rearrange("p a b -> p (a b)")
            q_load(0, 0, nc.sync)
            kv_load(0, 1, nc.gpsimd)
            kv_load(0, 2, nc.sync)
            kv_load(0, 3, nc.gpsimd)
            q_load(0, 1, nc.sync)

            # --- constants ---
            ident = persist.tile([128, DH], BF16, tag="ident")
            make_identity(nc, ident[0:DH, :])
            make_identity(nc, ident[DH : 2 * DH, :])
            # head-selector for the reciprocal broadcast: partition h*64
            # covers output partitions [h*64, (h+1)*64); rows 1-63 stay zero
            st2 = persist.tile([DH + 1, 128], F32, tag="st2")
            nc.vector.memset(st2[:], 0.0)
            nc.vector.memset(st2[0:1, 0:DH], 1.0)
            nc.vector.memset(st2[DH : DH + 1, DH : 2 * DH], 1.0)
            # reciprocal staging at partitions {0, 64}; rows 1-63 are zeroed
            # once and never written again, keeping the f32r contraction exact
            rcps = []
            for i in range(2):
                r = persist.tile([DH + 1, QTILE], F32, tag=f"rcp{i}")
                nc.vector.memset(r[:], 0.0)
                rcps.append(r)
            # prefetch the exp table set during the initial DMAs
            dummy = persist.tile([1, 8], F32, tag="dummy")
            nc.vector.memset(dummy[:], 0.0)
            nc.scalar.activation(dummy[:], dummy[:], Exp)

            qt_sb, kt_sb, vt_sb, vnat, outT = {}, {}, {}, {}, {}
            for b in range(B):
                qt_sb[b] = persist.tile([HD, SQ], BF16, tag=f"qt{b}", name=f"qt{b}")
                kt_sb[b] = persist.tile([HD, SKV], BF16, tag=f"kt{b}", name=f"kt{b}")
                vt_sb[b] = persist.tile([HD, SKV], BF16, tag=f"vt{b}", name=f"vt{b}")
                vnat[b] = persist.tile(
                    [128, HPC, NKT, DH + 1], BF16, tag=f"vn{b}", name=f"vn{b}"
                )
                outT[b] = persist.tile([HD, SQ], BF16, tag=f"ot{b}", name=f"ot{b}")
                nc.vector.memset(vnat[b][:, :, :, DH], 1.0)

            def _proj(dst, w_sb, xt, t0, w=PCHUNK):
                ps = miscp.tile([128, w], F32, tag="m", name="projp")
                for ko in range(KO):
                    nc.tensor.matmul(
                        ps[:],
                        w_sb[:, ko, :],
                        xt[:, ko, :],
                        start=(ko == 0),
                        stop=(ko == KO - 1),
                    )
                nc.vector.tensor_copy(dst[:, t0 : t0 + w], ps[:])

            def kproj(b, c):
                _proj(kt_sb[b], wk_sb, xts[(b, c)], c * PCHUNK)

            def qproj(b, c):
                _proj(qt_sb[b], wq_sb, xts.pop(("q", b, c)), c * PCHUNK)

            def vproj(b, c):
                """V projection for chunk c + PE-transpose into natural
                layout (k-tiles 4c..4c+3); frees the x chunk tile."""
                _proj(vt_sb[b], wv_sb, xts.pop((b, c)), c * PCHUNK)
                for h in range(HPC):
                    tp = miscp.tile([128, 4, DH], BF16, tag="m", name="vtp")
                    for i in range(4):
                        j = c * 4 + i
                        nc.tensor.transpose(
                            tp[:, i, :],
                            vt_sb[b][
                                h * DH : (h + 1) * DH,
                                j * KTILE : (j + 1) * KTILE,
                            ],
                            ident[h * DH : (h + 1) * DH, :],
                        )
                    nc.vector.tensor_copy(
                        vnat[b][:, h, c * 4 : (c + 1) * 4, 0:DH], tp[:]
                    )

            # --- attention ---
            acc_store = {0: {}, 1: {}}
            ucps = {}

            def attention(b, pre, post):
                """Flat software-pipelined attention over all (qt, j) steps.

                Scores for step t+LOOKAHEAD are emitted before PV of step t,
                so the PE always has score matmuls queued ahead of the
                exp/PV chain.  pre[t] hooks fire before scores(t); post[s]
                hooks fire right after step s's PV matmuls.
                """
                NT = NQT * NKT
                sps = {}
                accs = acc_store[b]

                def emit_scores(t):
                    qt, j = divmod(t, NKT)
                    q_sl = slice(qt * QTILE, (qt + 1) * QTILE)
                    k_sl = slice(j * KTILE, (j + 1) * KTILE)
                    sp = spsum.tile([128, HPC, QTILE], F32, tag="s", name="sp")
                    sps[t] = sp
                    for h in range(HPC):
                        h_sl = slice(h * DH, (h + 1) * DH)
                        nc.tensor.matmul(
                            sp[:, h, :],
                            kt_sb[b][h_sl, k_sl],
                            qt_sb[b][h_sl, q_sl],
                            start=True,
                            stop=True,
                        )

                def emit_tail(t):
                    qt, j = divmod(t, NKT)
                    sp = sps.pop(t)
                    ex = exps.tile([128, HPC, QTILE], BF16, tag="e", name="ex")
                    nc.scalar.activation(ex[:], sp[:], Exp, scale=SCALE)
                    if j == 0:
                        accs[qt] = [
                            accp.tile([128, QTILE], F32, tag="acc", name="acc")
                            for _ in range(HPC)
                        ]
                    for h in range(HPC):
                        nc.tensor.matmul(
                            accs[qt][h][0 : DH + 1, :],
                            vnat[b][:, h, j, :],
                            ex[:, h, :],
                            start=(j == 0),
                            stop=(j == NKT - 1),
                        )
                    if j == NKT - 1:
                        # drain the PSUM accumulators (incl. denominator row)
                        # to SBUF immediately; normalization is deferred
                        ucp = nrm.tile(
                            [DH + 1, HPC, QTILE], F32, tag="u", name="ucp"
                        )
                        acc2 = accs.pop(qt)
                        for h in range(HPC):
                            nc.vector.tensor_copy(
                                ucp[:, h, :], acc2[h][0 : DH + 1, :]
                            )
                        ucps[(b, qt)] = ucp

                for t in range(NT + LOOKAHEAD):
                    for fn in pre.get(t, ()):
                        fn()
                    if t < NT:
                        emit_scores(t)
                    if t >= LOOKAHEAD:
                        emit_tail(t - LOOKAHEAD)
                        for fn in post.get(t - LOOKAHEAD, ()):
                            fn()

            _flno = [0]
            _flst = {}

            def fstart(b, qt, eng=None):
                """Deferred normalization, part 1: repack the two denominator
                rows [1, 2, 512] -> [128, 8] with an SBUF->SBUF DMA so the
                DVE reciprocal runs wide (a [1, 512] reciprocal costs ~3.4us
                on the DVE; [128, 8] costs ~0.15us), then DMA back to row
                form at partitions {0, 64}.  fapply() runs ~4 steps later so
                the repack latency never stalls the PE.
                """
                ucp = ucps.pop((b, qt))
                rcp = rcps[_flno[0] % 2]
                _flno[0] += 1
                eng = eng or nc.gpsimd
                dpk = pkp.tile([128, HPC * QTILE // 128], F32, tag="dp")
                eng.dma_start(dpk[:], ucp[DH : DH + 1, :, :])
                rpk = pkp.tile([128, HPC * QTILE // 128], F32, tag="rp")
                nc.vector.reciprocal(rpk[:], dpk[:])
                for h in range(HPC):
                    eng.dma_start(
                        rcp[h * DH : h * DH + 1, :],
                        rpk[h * DH : (h + 1) * DH, :],
                    )
                _flst[(b, qt)] = (rcp, ucp)

            def fapply(b, qt):
                """Deferred normalization, part 2: broadcast both heads'
                reciprocals across partitions with a single f32r outer
                product against the head-selector, multiply into bf16 outT."""
                rcp, ucp = _flst.pop((b, qt))
                q_sl = slice(qt * QTILE, (qt + 1) * QTILE)
                bcp = miscp.tile([128, QTILE], F32, tag="m", name="bcp")
                nc.tensor.matmul(
                    bcp[:],
                    st2[:].bitcast(F32R),
                    rcp[:].bitcast(F32R),
                    start=True,
                    stop=True,
                )
                for h in range(HPC):
                    h_sl = slice(h * DH, (h + 1) * DH)
                    nc.vector.tensor_mul(
                        outT[b][h_sl, q_sl], ucp[0:DH, h, :], bcp[h_sl, :]
                    )

            def op(b, tt, split_copy=False):
                """Out-projection for one 128-token chunk + output DMA."""
                t_sl = slice(tt * 128, (tt + 1) * 128)
                ob = ost.tile([128, 2, 512], BF16, tag="o")
                for nt in range(DIM // 512):
                    ps = miscp.tile([128, 512], F32, tag="m", name="projo")
                    nc.tensor.matmul(
                        ps[:],
                        outT[b][:, t_sl],
                        wout_sb[:, nt * 512 : (nt + 1) * 512],
                        start=True,
                        stop=True,
                    )
                    if split_copy and nt % 2 == 0:
                        nc.scalar.copy(ob[:, nt, :], ps[:])
                    else:
                        nc.vector.tensor_copy(ob[:, nt, :], ps[:])
                nc.sync.dma_start(
                    out_d.ap()[
                        b * SQ + tt * 128 : b * SQ + (tt + 1) * 128, :
                    ].rearrange("t (n c) -> t n c", n=2),
                    ob[:],
                )

            # --- startup: first projections, then attention begins ---
            kproj(0, 0)
            qproj(0, 0)

            L = lambda fn, *a, **k: (lambda: fn(*a, **k))

            pre0 = {
                4: [L(kproj, 0, 1)],
                6: [L(vproj, 0, 0)],
                8: [L(kproj, 0, 2)],
                12: [L(kproj, 0, 3)],
            }
            post0 = {
                3: [L(vproj, 0, 1)],
                6: [L(q_load, 0, 2)],
                7: [L(vproj, 0, 2)],
                8: [L(kv_load, 1, 0, nc.scalar)],
                9: [L(qproj, 0, 1)],
                11: [L(vproj, 0, 3)],
                14: [L(q_load, 0, 3)],
                15: [L(kproj, 1, 0)],
                17: [L(qproj, 0, 2)],
                19: [L(kv_load, 1, 1, nc.scalar), L(q_load, 1, 0)],
                23: [L(fstart, 0, 0)],
                25: [L(kproj, 1, 1)],
                27: [L(fapply, 0, 0)],
                28: [L(op, 0, 0)],
                29: [L(op, 0, 1), L(qproj, 0, 3)],
                30: [L(op, 0, 2)],
                31: [L(op, 0, 3)],
                33: [L(kv_load, 1, 2), L(q_load, 1, 1)],
                35: [L(vproj, 1, 0)],
                37: [L(qproj, 1, 0)],
                39: [L(fstart, 0, 1)],
                41: [L(kproj, 1, 2)],
                43: [L(fapply, 0, 1), L(kv_load, 1, 3)],
                44: [L(op, 0, 4)],
                45: [L(op, 0, 5), L(vproj, 1, 1)],
                46: [L(op, 0, 6)],
                47: [L(op, 0, 7)],
                49: [L(q_load, 1, 2)],
                51: [L(qproj, 1, 1)],
                53: [L(kproj, 1, 3)],
                55: [L(fstart, 0, 2)],
                57: [L(vproj, 1, 2), L(q_load, 1, 3)],
                58: [L(vproj, 1, 3)],
                59: [L(fapply, 0, 2)],
                60: [L(op, 0, 8)],
                61: [L(op, 0, 9), L(qproj, 1, 2)],
                62: [L(op, 0, 10)],
                63: [L(op, 0, 11), L(qproj, 1, 3)],
            }
            attention(0, pre0, post0)

            post1 = {
                0: [L(fstart, 0, 3)],
                4: [L(fapply, 0, 3)],
                5: [L(op, 0, 12)],
                7: [L(op, 0, 13)],
                9: [L(op, 0, 14)],
                11: [L(op, 0, 15)],
                17: [L(fstart, 1, 0)],
                21: [L(fapply, 1, 0)],
                22: [L(op, 1, 0)],
                24: [L(op, 1, 1)],
                26: [L(op, 1, 2)],
                28: [L(op, 1, 3)],
                33: [L(fstart, 1, 1)],
                37: [L(fapply, 1, 1)],
                38: [L(op, 1, 4)],
                40: [L(op, 1, 5)],
                42: [L(op, 1, 6)],
                44: [L(op, 1, 7)],
                49: [L(fstart, 1, 2)],
                55: [L(fapply, 1, 2)],
                56: [L(op, 1, 8)],
                58: [L(op, 1, 9)],
            }
            attention(1, {}, post1)
            fstart(1, 3, nc.sync)
            op(1, 10)
            op(1, 11)
            fapply(1, 3)
            op(1, 12, split_copy=True)
            op(1, 13, split_copy=True)
            op(1, 14, split_copy=True)
            op(1, 15, split_copy=True)

    nc.compile()
    return nc


def make_in_maps(x_q, x_kv, W_qkv, W_out):
    x_q = np.asarray(x_q, dtype=np.float32)
    x_kv = np.asarray(x_kv, dtype=np.float32)
    W_qkv = np.asarray(W_qkv, dtype=np.float32)
    W_out = np.asarray(W_out, dtype=np.float32)

    def chunk_tile(x):
        # [TOK, DIM] -> [n_chunks, 128, KO, PCHUNK] with D = ko*128 + p
        xt = x.reshape(TOK, DIM).T.reshape(KO, 128, TOK // PCHUNK, PCHUNK)
        return np.ascontiguousarray(xt.transpose(2, 1, 0, 3)).astype(BF)

    def w_tile(w):
        # [DIM, HD] -> [128, KO, HD] partition-major contiguous
        return np.ascontiguousarray(
            w.reshape(KO, 128, HD).transpose(1, 0, 2)
        ).astype(BF)

    xqt = chunk_tile(x_q)
    xkvt = chunk_tile(x_kv)

    in_maps = []
    for c in range(N_CORES):
        cs = slice(c * HD, (c + 1) * HD)
        wk = w_tile(W_qkv[:, 1024:][:, cs])
        wq = w_tile(W_qkv[:, cs])
        wv = w_tile(W_qkv[:, 2048:][:, cs])
        wout = np.ascontiguousarray(W_out[cs, :]).astype(BF)
        in_maps.append(
            {
                "xqt": xqt,
                "xkvt": xkvt,
                "wkq": np.ascontiguousarray(np.stack([wk, wq], axis=1)),
                "wvo": np.ascontiguousarray(
                    np.stack([wv, wout.reshape(128, KO, HD)], axis=1)
                ),
            }
        )
    return in_maps


def combine(partials, b_out):
    """Sum the 8 per-core partial projections and add the bias."""
    acc = np.zeros((TOK, DIM), dtype=np.float32)
    for p in partials:
        acc += np.asarray(p, dtype=np.float32)
    acc += np.asarray(b_out, dtype=np.float32)
    return acc.reshape(B, SQ, DIM)


_STATE = {}


def _get_nc():
    if "nc" not in _STATE:
        _STATE["nc"] = build()
    return _STATE["nc"]


def run(x_q, x_kv, W_qkv, W_out, b_out, trace=False):
    nc = _get_nc()
    in_maps = make_in_maps(x_q, x_kv, W_qkv, W_out)
    res = run_bass_kernel_spmd(nc, in_maps, list(range(N_CORES)), trace=trace)
    out = combine([r["out"] for r in res.results], b_out)
    return out, res


def kernel(x_q, x_kv, W_qkv, W_out, b_out):
    out, _ = run(x_q, x_kv, W_qkv, W_out, b_out, trace=False)
    return out
